# revision 43
# baseline (speedup 1.0000x reference)
"""Grouped linear (MoE routing) Trainium2 kernel.

y[t] = x[t] @ weight[g_t] + bias[g_t],  g_t = group_indices[t]

Data-parallel over 8 cores (8192 tokens each), weights replicated.
Per core:
  1. On-device counting sort of tokens by group: one wide [P, G, F] mask +
     a single free-dim scan give within-partition per-group ranks fused
     with per-partition group-prefix totals; a strict-lower-triangular
     bf16 matmul gives cross-partition prefixes.
  2. dest[t] (slot in the group-blocked order, blocks statically sized
     from host-computed max counts) is scattered token-id-wise into FOUR
     independent permutation tables in DRAM (one per dma_scatter_add on
     its own SWDGE queue: queue_num selects the Q7 core pair that
     generates descriptors, and separate tables avoid the Tile W-W
     serialization, so the four descgens overlap).
  3. The tables are reloaded in wrap-16 layout (gather indices; pads
     clamp to row 0) and per-tile column layout (output scatter offsets;
     pads -> OOB sentinel), summed on DVE. The reloads are split into a
     head segment (first 2048 slots) that unblocks the first gathers /
     GEMM tiles early, and a tail that hides behind the GEMM.
  4. dma_gather(transpose=True) on round-robin SWDGE queues 1-3 fetches
     x rows in sorted order directly as contraction-major tiles.
  5. Grouped GEMM: per 128-token tile, 8 K-chunks of (K=128, M=128)
     stationary loads, each streaming both N=512 chunks of the group
     weights; weights stream through SBUF double-buffered.
  6. DVE fuses bias add (bf16, partition_broadcast) with PSUM->SBUF copy
     into bf16 y tiles; indirect_dma_start scatters rows to one of four
     round-robin output tensors (avoids a W-W receipt chain on one
     tensor), skipping pads via bounds_check. Host sums the four outputs
     and upcasts to f32 (the reference output is bf16-rounded anyway).

Input-layout notes: gi is passed duplicated ([2*TOK]) and gbase/pbase
ride one padded [P, 128] f32 tensor so every DMA moves >= 512 B per
partition (sub-512 B transfers pay a read-modify-write penalty and
~10 us completion latency under load).
"""

import sys

import numpy as np

sys.path.insert(0, "/opt/trn_rl_repo")

from concourse import bacc, bass, mybir, tile  # noqa: E402

N_CORES = 8
BATCH = 65536
TOK = BATCH // N_CORES  # tokens per core
DIN = 1024
DOUT = 1024
NG = 8
P = 128
TPF = TOK // P  # 64 columns, token t = (t % 128, t // 128)

FP32 = mybir.dt.float32
BF16 = mybir.dt.bfloat16
I32 = mybir.dt.int32
I16 = mybir.dt.int16

SENTINEL = 99999  # > TOK-1: skipped by bounds_check on output scatter
OFFV = 16384
E = 64  # perm-table row stride in f32 (256 B dma_scatter_add stride min)
EW = 16  # written payload per token (64 B; stride stays 256 B)
SCH = 2048  # indices per scatter_add (8192 in one call overflows the
# SWDGE prep FIFO and wedges the exec unit)
GCH = 512  # slots per gather chunk (1024 idxs overflows the
# single-packet SWDGE gather: 64 descs/lane kills the exec unit)
NOUT = 4  # round-robin output tensors
HCUT = 2048  # head/tail reload split (slots)

Alu = mybir.AluOpType


def build_kernel(cap):
    """cap[g] = static slot capacity of group g (multiple of 128, >=
    per-core count of group g on every core)."""
    cap = [int(c) for c in cap]
    assert all(c % P == 0 for c in cap) and sum(cap) % P == 0
    nslots = sum(cap)
    ntiles = nslots // P
    cols16 = nslots // 16
    hcols = HCUT // 16  # 128
    htiles = HCUT // P  # 16
    assert nslots > HCUT

    tile_group = []
    for g in range(NG):
        tile_group += [g] * (cap[g] // P)

    nc = bacc.Bacc(
        "TRN2",
        target_bir_lowering=False,
        debug=False,
        num_devices=N_CORES,
        num_swdge_queues=4,
    )

    x_d = nc.dram_tensor("x", [TOK, DIN], BF16, kind="ExternalInput").ap()
    gi_d = nc.dram_tensor("gi", [2 * TOK], I32, kind="ExternalInput").ap()
    w_d = nc.dram_tensor("w", [NG, DIN, DOUT], BF16, kind="ExternalInput").ap()
    b_d = nc.dram_tensor("b", [NG, DOUT], BF16, kind="ExternalInput").ap()
    gb_d = nc.dram_tensor("gb", [P, P], FP32, kind="ExternalInput").ap()
    out_d = [
        nc.dram_tensor(f"out{o}", [TOK, DOUT], BF16, kind="ExternalOutput").ap()
        for o in range(NOUT)
    ]

    with tile.TileContext(nc) as tc:
        with (
            tc.tile_pool(name="sbuf", bufs=1) as sb,
            tc.tile_pool(name="bpool", bufs=2) as bpool,
            tc.tile_pool(name="wpool", bufs=2) as wpool,
            tc.tile_pool(name="gpool", bufs=11) as gpool,
            tc.tile_pool(name="ypool", bufs=3) as ypool,
            tc.tile_pool(name="psum", bufs=4, space="PSUM") as psum,
            tc.tile_pool(name="psum_small", bufs=1, space="PSUM") as psum_s,
            tc.tile_pool(name="psum_t", bufs=2, space="PSUM") as psum_t,
            tc.tile_pool(name="segpool", bufs=4) as segpool,
            tc.tile_pool(name="dram", bufs=1, space="DRAM") as dram,
        ):
            # gi first (duplicated to 512 B/partition): the whole metadata
            # chain hangs off it.
            gi2 = sb.tile([P, 2 * TPF], I32, tag="gi")
            nc.sync.dma_start(out=gi2[:], in_=gi_d.rearrange("(f p) -> p f", p=P))
            gi_sb = gi2[:, 0:TPF]
            gb2 = sb.tile([P, P], FP32, tag="gb")
            nc.sync.dma_start(out=gb2[:], in_=gb_d[:])
            gb_sb = gb2[:, 0:NG]  # gbase
            pbf = gb2[:, NG : NG + 1]  # pbase (f32)

            # ---------- gi-independent prep (stock-lib gpsimd ops first) ----------
            lt_i = sb.tile([P, P], I32, tag="lt_i")
            nc.gpsimd.iota(lt_i[:], pattern=[[-1, P]], base=0, channel_multiplier=1)
            gvec = sb.tile([P, NG, TPF], I32, tag="gvec")
            nc.gpsimd.iota(
                gvec[:], pattern=[[1, NG], [0, TPF]], base=0, channel_multiplier=0
            )
            vi = sb.tile([P, 2, 8], I32, tag="vi")
            nc.gpsimd.iota(
                vi[:], pattern=[[16, 2], [1024, 8]], base=OFFV, channel_multiplier=0
            )

            # zero-source for the table zeroing FIRST (on the scalar engine
            # so the vector chain isn't delayed): the 9 MB of zero writes
            # gate the scatters, so their DMAs must issue early
            zt = sb.tile([P, nslots * E // P], FP32, tag="zt")
            nc.scalar.memzero(zt[:])

            # scatter-feeding vector work next so the warm + real scatters
            # dispatch as early as possible
            warm_v = sb.tile([P, 1, EW], FP32, tag="warm_v")
            nc.vector.memset(warm_v[:], 0.0)
            warm_i = sb.tile([P, 8], I16, tag="warm_i")
            nc.vector.memset(warm_i[:], 0)
            vf = sb.tile([P, 16], FP32, tag="vf")
            nc.vector.tensor_copy(out=vf[:], in_=vi[:].rearrange("p a b -> p (a b)"))
            nc.vector.tensor_scalar(
                out=vf[:], in0=vf[:], scalar1=pbf, scalar2=None, op0=Alu.add
            )
            vks = []
            for k in range(4):
                vk = sb.tile([P, 16, EW], FP32, tag=f"vk{k}")
                nc.vector.tensor_scalar(
                    out=vk[:],
                    in0=vf[:, :, None].to_broadcast([P, 16, EW]),
                    scalar1=float(32 * k),
                    scalar2=None,
                    op0=Alu.add,
                )
                vks.append(vk)

            # Prewarm the Q7 scatter_add ext-isa lib (~18us IRAM load)
            # during the vector phase.
            scratch = dram.tile([P, E], FP32, tag="scratch")
            nc.gpsimd.dma_scatter_add(
                scratch[:, 0:EW], warm_v[:], warm_i[:], P, P, EW, elem_step=E
            )

            lt = sb.tile([P, P], BF16, tag="lt")
            nc.vector.tensor_scalar(
                out=lt[:], in0=lt_i[:], scalar1=0, scalar2=None, op0=Alu.is_lt
            )
            idf = sb.tile([P, P], FP32, tag="idf")  # identity, for PE transpose
            nc.vector.tensor_scalar(
                out=idf[:], in0=lt_i[:], scalar1=0, scalar2=None, op0=Alu.is_equal
            )
            zeros = sb.tile([P, NG * TPF], FP32, tag="zeros")
            nc.vector.memset(zeros[:], 0.0)

            # Zero the four perm tables (pads must read back 0). Quartered
            # DMAs: one huge transfer would hold its HWDGE queue ~30 us.
            ptabs = []
            for k in range(4):
                pt = dram.tile([nslots, E], FP32, tag=f"ptab{k}")
                qn = nslots * E // P // 4
                flat = pt[:].rearrange("(p f) e -> p (f e)", p=P)
                for q in range(4):
                    eng = nc.sync if (k + q) % 2 == 0 else nc.scalar
                    eng.dma_start(
                        out=flat[:, q * qn : (q + 1) * qn],
                        in_=zt[:, q * qn : (q + 1) * qn],
                    )
                ptabs.append(pt)

            # bias rows staged early (2 KB single-partition loads are fine)
            bgs = []
            for g in range(NG):
                bg = bpool.tile([1, DOUT], BF16, tag="b16")
                nc.scalar.dma_start(out=bg[:], in_=b_d[g : g + 1, :])
                bgs.append(bg)
            ones1 = sb.tile([1, P], BF16, tag="ones1")
            nc.vector.memset(ones1[:], 1.0)

            # ---------------- routing metadata ----------------
            masks = sb.tile([P, NG, TPF], FP32, tag="masks")
            nc.vector.tensor_tensor(
                out=masks[:],
                in0=gi_sb[:, None, :].to_broadcast([P, NG, TPF]),
                in1=gvec[:],
                op=Alu.is_equal,
            )
            bigscan = sb.tile([P, NG, TPF], FP32, tag="bigscan")
            nc.vector.tensor_tensor_scan(
                out=bigscan[:].rearrange("p g t -> p (g t)"),
                data0=masks[:].rearrange("p g t -> p (g t)"),
                data1=zeros[:],
                initial=0.0,
                op0=Alu.add,
                op1=Alu.add,
            )
            ct1 = sb.tile([P, NG], FP32, tag="ct1")
            nc.vector.memset(ct1[:, 0:1], 1.0)
            nc.vector.tensor_scalar(
                out=ct1[:, 1:NG],
                in0=bigscan[:, 0 : NG - 1, TPF - 1],
                scalar1=1.0,
                scalar2=None,
                op0=Alu.add,
            )
            tg = sb.tile([P, NG], BF16, tag="tg")
            nc.vector.tensor_tensor(
                out=tg[:], in0=bigscan[:, :, TPF - 1], in1=ct1[:], op=Alu.subtract
            )
            nc.vector.tensor_scalar(
                out=tg[:], in0=tg[:], scalar1=-1.0, scalar2=None, op0=Alu.subtract
            )
            e_ps = psum_s.tile([P, NG], FP32, tag="E")
            nc.tensor.matmul(out=e_ps[:], lhsT=lt[:], rhs=tg[:], start=True, stop=True)
            cpg = sb.tile([P, NG], FP32, tag="cpg")
            nc.vector.tensor_tensor(out=cpg[:], in0=e_ps[:], in1=ct1[:], op=Alu.subtract)
            nc.vector.tensor_tensor(out=cpg[:], in0=cpg[:], in1=gb_sb, op=Alu.add)

            tmpw = sb.tile([P, NG, TPF], FP32, tag="tmpw")
            nc.vector.tensor_tensor(
                out=tmpw[:],
                in0=cpg[:, :, None].to_broadcast([P, NG, TPF]),
                in1=bigscan[:],
                op=Alu.add,
            )
            nc.vector.tensor_tensor(
                out=tmpw[:], in0=tmpw[:], in1=masks[:], op=Alu.mult
            )
            d4 = sb.tile([P, 4, TPF], FP32, tag="d4")
            nc.vector.tensor_tensor(
                out=d4[:], in0=tmpw[:, 0:4, :], in1=tmpw[:, 4:8, :], op=Alu.add
            )
            d2 = sb.tile([P, 2, TPF], FP32, tag="d2")
            nc.vector.tensor_tensor(
                out=d2[:], in0=d4[:, 0:2, :], in1=d4[:, 2:4, :], op=Alu.add
            )
            dest = sb.tile([P, TPF], FP32, tag="dest")
            nc.vector.tensor_tensor(
                out=dest[:], in0=d2[:, 0, :], in1=d2[:, 1, :], op=Alu.add
            )
            dest16 = sb.tile([P, TPF], I16, tag="dest16")
            nc.vector.tensor_copy(out=dest16[:], in_=dest[:])

            # idxw[q, 64s + f] = dest16[16s + q, f], replicated to 128 parts
            idxw = sb.tile([P, TOK // 16], I16, tag="idxw")
            for s in range(8):
                eng = nc.sync if s % 2 == 0 else nc.scalar
                eng.dma_start(
                    out=idxw[0:16, 64 * s : 64 * (s + 1)],
                    in_=dest16[16 * s : 16 * (s + 1), :],
                )
            for rep in range(1, 8):
                eng = nc.sync if rep % 2 == 0 else nc.scalar
                eng.dma_start(
                    out=idxw[rep * 16 : (rep + 1) * 16, :], in_=idxw[0:16, :]
                )

            # four concurrent token-id scatters (own table + own Q7 pair)
            for k in range(TOK // SCH):
                nc.gpsimd.dma_scatter_add(
                    ptabs[k][:, 0:EW],
                    vks[k][:],
                    idxw[:, 128 * k : 128 * (k + 1)],
                    SCH,
                    SCH,
                    EW,
                    elem_step=E,
                    queue_num=k,
                )
            # Prewarm the gather ext-isa lib: the MPC swap waits for the
            # scatter descgens to drain, then the ~18us IRAM load overlaps
            # the head reloads.
            warm_g = gpool.tile([P, DIN // P, P], BF16, tag="g")
            nc.gpsimd.dma_gather(
                warm_g[:], x_d[:], warm_i[:], P, P, DIN, transpose=True, queue_num=1
            )

            # bias broadcast via K=1 PE matmuls (keeps GpSimd free for the
            # scatter/gather stream): bias_rep[p, g, :] = 1 * bias[g, :]
            bias_rep = sb.tile([P, NG, DOUT], BF16, tag="bias_rep")
            for g in range(NG):
                for jc in range(2):
                    bp = psum.tile([P, 512], FP32, tag="acc")
                    nc.tensor.matmul(
                        out=bp[:],
                        lhsT=ones1[:],
                        rhs=bgs[g][:, jc * 512 : (jc + 1) * 512],
                        start=True,
                        stop=True,
                    )
                    nc.vector.tensor_copy(
                        out=bias_rep[:, g, jc * 512 : (jc + 1) * 512], in_=bp[:]
                    )

            # ------- reloads: contiguous 512 KB segments + PE-transpose -------
            # Segment of SEG=2048 slots: partition c holds rows
            # [base + 16c, base + 16c + 16), so the [128, 16] column-0 view
            # PE-transposes to [16, 128] = exactly the wrap-16 layout.
            # PSUM accumulation over the four tables sums them for free.
            idx16 = sb.tile([P, cols16], I16, tag="idx16")
            yoff = sb.tile([P, ntiles], I32, tag="yoff")
            SEG = 2048
            seg_bases = list(range(0, nslots - SEG + 1, SEG))
            if seg_bases[-1] + SEG < nslots:
                seg_bases.append(nslots - SEG)  # overlap is idempotent

            n_chunks = (nslots + GCH - 1) // GCH
            gtiles = []

            def emit_gather(ch):
                s0 = ch * GCH
                n = min(GCH, nslots - s0)
                gt = gpool.tile([P, DIN // P, n], BF16, tag="g")
                # first chunks use all 4 Q7 pairs (no output scatters yet);
                # later ones keep pair 0 free for the indirect scatters
                qn = ch % 4 if ch < 8 else 1 + ch % 3
                nc.gpsimd.dma_gather(
                    gt[:],
                    x_d[:],
                    idx16[:, s0 // 16 : (s0 + n) // 16],
                    n,
                    n,
                    DIN,
                    transpose=True,
                    queue_num=qn,
                )
                gtiles.append(gt)

            w_sb = {}

            def emit_weight(g):
                wt = wpool.tile([P, DIN // P, DOUT], BF16, tag="w")
                nc.scalar.dma_start(
                    out=wt[:], in_=w_d[g].rearrange("(c p) j -> p c j", p=P)
                )
                w_sb[g] = wt

            emitted = 0

            def emit_segment(si, base):
                nonlocal emitted
                tp = psum_t.tile([16, SEG // 16], FP32, tag="T")
                for k in range(4):
                    pseg = segpool.tile([P, SEG // P, E], FP32, tag="pseg")
                    eng = nc.sync if (si + k) % 2 == 0 else nc.scalar
                    eng.dma_start(
                        out=pseg[:],
                        in_=ptabs[k][base : base + SEG, :].rearrange(
                            "(c q) e -> c q e", q=16
                        ),
                    )
                    nc.tensor.matmul(
                        out=tp[:],
                        lhsT=pseg[:, :, 0],
                        rhs=idf[:],
                        is_transpose=True,
                        start=(k == 0),
                        stop=(k == 3),
                    )
                c0 = base // 16
                ta = sb.tile([16, SEG // 16], FP32, tag="segA")
                nc.vector.tensor_scalar(
                    out=ta[:], in0=tp[:], scalar1=float(OFFV),
                    scalar2=float(OFFV), op0=Alu.max, op1=Alu.subtract,
                )
                nc.vector.tensor_copy(
                    out=idx16[0:16, c0 : c0 + SEG // 16], in_=ta[:]
                )
                for rep in range(1, 8):
                    eng = nc.sync if rep % 2 == 0 else nc.scalar
                    eng.dma_start(
                        out=idx16[rep * 16 : (rep + 1) * 16, c0 : c0 + SEG // 16],
                        in_=idx16[0:16, c0 : c0 + SEG // 16],
                    )
                # gathers whose slots this segment completes (before the
                # yoff work: they're the critical consumers)
                while (
                    emitted < n_chunks
                    and min((emitted + 1) * GCH, nslots) <= base + SEG
                ):
                    emit_gather(emitted)
                    emitted += 1
                # yoff: yoff[16m + q, t0 + t'] = T[q, 8t' + m]
                yb = sb.tile([16, SEG // 16], FP32, tag="segB")
                nc.vector.tensor_scalar(
                    out=yb[:], in0=tp[:], scalar1=float(OFFV), scalar2=None,
                    op0=Alu.subtract,
                )
                ym = sb.tile([16, SEG // 16], FP32, tag="segM")
                nc.vector.tensor_scalar(
                    out=ym[:], in0=yb[:], scalar1=0.0,
                    scalar2=float(SENTINEL), op0=Alu.is_lt, op1=Alu.mult,
                )
                ybi = sb.tile([16, SEG // 16], I32, tag="segBI")
                nc.vector.tensor_tensor(
                    out=ybi[:], in0=yb[:], in1=ym[:], op=Alu.add
                )
                t0 = base // P
                for m in range(8):
                    eng = nc.sync if m % 2 == 0 else nc.scalar
                    eng.dma_start(
                        out=yoff[16 * m : 16 * (m + 1), t0 : t0 + SEG // P],
                        in_=ybi[:].rearrange("q (t m) -> q t m", m=8)[:, :, m],
                    )

            # Segments 0-1 before the GEMM; later segments interleave into
            # the tile loop below (the Tensor queue is in-order, so a late
            # segment's PE-transpose emitted before the GEMM would stall
            # every matmul behind its table load).
            emit_segment(0, seg_bases[0])
            emit_weight(0)
            emit_weight(1)
            emit_segment(1, seg_bases[1])
            for g in range(2, NG):
                emit_weight(g)
            next_seg = 2

            for t in range(ntiles):
                if next_seg < len(seg_bases) and t == 8 * (next_seg - 1):
                    emit_segment(next_seg, seg_bases[next_seg])
                    next_seg += 1
                g = tile_group[t]
                ch, off = divmod(t * P, GCH)
                gt = gtiles[ch]
                ps0 = psum.tile([P, 512], FP32, tag="acc")
                ps1 = psum.tile([P, 512], FP32, tag="acc")
                for ic in range(DIN // P):
                    first = ic == 0
                    last = ic == DIN // P - 1
                    nc.tensor.matmul(
                        out=ps0[:],
                        lhsT=gt[:, ic, off : off + P],
                        rhs=w_sb[g][:, ic, 0:512],
                        start=first,
                        stop=last,
                    )
                    nc.tensor.matmul(
                        out=ps1[:],
                        lhsT=gt[:, ic, off : off + P],
                        rhs=w_sb[g][:, ic, 512:1024],
                        start=first,
                        stop=last,
                    )
                y_st = ypool.tile([P, DOUT], BF16, tag="y")
                nc.vector.tensor_tensor(
                    out=y_st[:, 0:512],
                    in0=ps0[:],
                    in1=bias_rep[:, g, 0:512],
                    op=Alu.add,
                )
                nc.vector.tensor_tensor(
                    out=y_st[:, 512:1024],
                    in0=ps1[:],
                    in1=bias_rep[:, g, 512:1024],
                    op=Alu.add,
                )
                nc.gpsimd.indirect_dma_start(
                    out=out_d[t % NOUT][:],
                    out_offset=bass.IndirectOffsetOnAxis(
                        ap=yoff[:, t : t + 1], axis=0
                    ),
                    in_=y_st[:],
                    in_offset=None,
                    bounds_check=TOK - 1,
                    oob_is_err=False,
                )

    nc.compile()
    return nc


def _plan_caps(gi: np.ndarray) -> np.ndarray:
    counts = np.zeros((N_CORES, NG), dtype=np.int64)
    for c in range(N_CORES):
        counts[c] = np.bincount(gi[c * TOK : (c + 1) * TOK], minlength=NG)
    mx = counts.max(axis=0)
    return ((mx + P - 1) // P) * P


_PBASE = (np.arange(P) % 16 + 128 * (np.arange(P) // 16)).astype(np.float32)

LAST_RESULTS = None  # stashed BassKernelResults for external profiling


def kernel(x, weight, bias, group_indices):
    global LAST_RESULTS
    from concourse.bass_utils import run_bass_kernel_spmd

    x = np.asarray(x)
    weight = np.asarray(weight)
    bias = np.asarray(bias)
    gi = np.ascontiguousarray(np.asarray(group_indices, dtype=np.int32))

    cap = _plan_caps(gi)
    nc = build_kernel(cap)
    gbase = np.cumsum([0] + [int(c) for c in cap])[:-1].astype(np.float64)
    gb = np.zeros((P, P), dtype=np.float32)
    gb[:, 0:NG] = gbase[None, :]
    gb[:, NG] = _PBASE
    gb = np.ascontiguousarray(gb)

    in_maps = []
    for c in range(N_CORES):
        gic = gi[c * TOK : (c + 1) * TOK]
        in_maps.append(
            {
                "x": np.ascontiguousarray(x[c * TOK : (c + 1) * TOK]),
                "gi": np.ascontiguousarray(np.concatenate([gic, gic])),
                "w": weight,
                "b": bias,
                "gb": gb,
            }
        )
    res = run_bass_kernel_spmd(nc, in_maps, core_ids=list(range(N_CORES)))
    LAST_RESULTS = res
    outs = []
    for c in range(N_CORES):
        acc = res.results[c]["out0"].astype(np.float32)
        for o in range(1, NOUT):
            acc += res.results[c][f"out{o}"].astype(np.float32)
        outs.append(acc)
    return np.concatenate(outs, axis=0)
